# revision 3
# baseline (speedup 1.0000x reference)
"""Trainium2 Bass kernel for BoundaryLoss (data-parallel over batch).

Math (per batch sample b):
  mask  = boundary mask of target = (maxpool5x5(t) != minpool5x5(t)) with
          cv2-style clipped windows (OOB ignored).  Equals the reference's
          per-class dilate/erode union because a 5x5 window is non-uniform
          iff some class boundary passes through it.
  ce    = logsumexp_c(pred) - pred[t]
  wsum  = sum(mask * ce);  msum = sum(mask)
  per_sample = msum > 0 ? wsum/max(msum,1) : wsum/(H*W);  out = mean_b

Device algorithm (one sample per core), all in "layout B" (partition p
holds rows 4p..4p+3, giving 8KB-contiguous DMA runs for pred):
  - pred streams in CHUNK-class pieces on the sync HWDGE ring from t=0;
    target loads concurrently on the scalar HWDGE ring.
  - S = sum_c exp(pred_c): exp on ACT (fp16 out), classes pair-summed on
    DVE, accumulated per-pixel by identity-matmul into PSUM (4 banks).
  - G = exp(pred)[t] gathered per class by ONE fused stt
    (t==c)*e_c on DVE, accumulated into the other 4 PSUM banks.
  - boundary mask concurrently in layout B: horizontal 5-pools over a
    stacked [t; -t] fp16 tile (min-pool = -maxpool(-t)), cross-partition
    +-1 row shifts via two SBUF->SBUF DMAs on the gpsimd ring, then a
    sliding-max chain; mask = (vmax != vmin).  Stages are interleaved
    between class chunks to keep DVE's static stream dependency-ready.
  - finals per row group: ln(S),ln(G) on ACT (PSUM src), masked sums via
    stt accum into per-row accumulator columns; ones-matmul partition
    reduce; DMA out [1,16].
Host combines the per-core outputs.
"""

import numpy as np

B = 8
C = 21
H = 512
W = 512
N_CORES = 8
CHUNK = 2  # pred planes per DMA
PW = 520  # padded width of pooling buffers; data cols [2, 514)
G4 = 4  # row groups per partition (H = 128 * G4)

_CACHE = {}


def _build_nc():
    from contextlib import ExitStack

    import concourse.bacc as bacc
    import concourse.tile as tile
    from concourse import mybir
    from concourse.masks import make_identity

    dt = mybir.dt
    Alu = mybir.AluOpType
    Act = mybir.ActivationFunctionType

    nc = bacc.Bacc("TRN2", target_bir_lowering=False, debug=False,
                   num_devices=N_CORES)

    pred = nc.dram_tensor("pred", [C, H, W], dt.float32, kind="ExternalInput")
    target = nc.dram_tensor("target", [H, W], dt.int32, kind="ExternalInput")
    out = nc.dram_tensor("out", [1, 16], dt.float32, kind="ExternalOutput")

    with tile.TileContext(nc) as tc, ExitStack() as ctx:
        consts = ctx.enter_context(tc.tile_pool(name="consts", bufs=1))
        keep = ctx.enter_context(tc.tile_pool(name="keep", bufs=1))
        mp = ctx.enter_context(tc.tile_pool(name="maskpool", bufs=1))
        ms = ctx.enter_context(tc.tile_pool(name="maskscratch", bufs=1))
        ppool = ctx.enter_context(tc.tile_pool(name="pp", bufs=3))
        epool = ctx.enter_context(tc.tile_pool(name="ep", bufs=3))
        e2pool = ctx.enter_context(tc.tile_pool(name="e2p", bufs=2))
        opool = ctx.enter_context(tc.tile_pool(name="op", bufs=3))
        fin = ctx.enter_context(tc.tile_pool(name="fin", bufs=1))
        sgp = ctx.enter_context(tc.tile_pool(name="sgpsum", bufs=1,
                                             space="PSUM"))

        ident = consts.tile([128, 128], dt.float16)
        make_identity(nc, ident)
        ones = consts.tile([128, 1], dt.float32)
        nc.gpsimd.memset(ones, 1.0)
        warm = consts.tile([128, 512], dt.float16)
        nc.gpsimd.memset(warm, 0.0)
        # accumulator columns: [w1_r0..3 | l2_r0..3 | msum | pad x3]
        stc = consts.tile([128, 12], dt.float32)
        nc.vector.memset(stc, 0.0)

        # PSUM: S and G accumulators, 4 banks each
        s_ps = sgp.tile([128, G4, W], dt.float32, tag="s")
        g_ps = sgp.tile([128, G4, W], dt.float32, tag="g")

        # ---------------- mask tiles (layout B) ----------------
        # sign 0 = +t (max pool), sign 1 = -t (min pool via max)
        xb = mp.tile([128, 2, G4, PW], dt.float16)
        m2 = ms.tile([128, 2, G4, PW], dt.float16, tag="m2")
        m4 = ms.tile([128, 2, G4, PW], dt.float16, tag="m4")
        hb = mp.tile([128, 2, G4, W], dt.float16)
        sh_dn = mp.tile([128, 2, 2, W], dt.float16)  # rows {2,3} of p-1
        sh_up = mp.tile([128, 2, 2, W], dt.float16)  # rows {0,1} of p+1
        v = mp.tile([128, 2, G4, W], dt.float16)
        va = ms.tile([128, 2, W], dt.float16, tag="va")
        vb = ms.tile([128, 2, W], dt.float16, tag="vb")
        vm4 = ms.tile([128, 2, W], dt.float16, tag="vm4")
        vm012 = ms.tile([128, 2, W], dt.float16, tag="vm012")
        vm123 = ms.tile([128, 2, W], dt.float16, tag="vm123")
        vx1 = ms.tile([128, 2, W], dt.float16, tag="vx1")
        maskb = keep.tile([128, G4, W], dt.float16)
        t32 = mp.tile([128, G4, W], dt.int32)
        tb = xb[:, 0, :, 2:2 + W]  # fp16 target, contiguous-row alias

        # ---------------- early loads / init ----------------
        nc.scalar.dma_start(
            out=t32, in_=target.ap().rearrange("(p r) w -> p r w", p=128))
        nc.gpsimd.memset(xb[:, 0, :, 0:2], -1.0)
        nc.gpsimd.memset(xb[:, 0, :, 2 + W:PW], -1.0)
        nc.gpsimd.memset(xb[:, 1, :, 0:2], -99.0)
        nc.gpsimd.memset(xb[:, 1, :, 2 + W:PW], -99.0)

        # PE warmup into the G bank (discarded by class-0 start=True)
        for _ in range(10):
            nc.tensor.matmul(g_ps[:, 0, :], ident, warm, start=True,
                             stop=True)

        # cast target to fp16 (+ negated copy) before first class stt
        nc.vector.tensor_copy(out=tb, in_=t32)
        nc.vector.tensor_scalar(out=xb[:, 1, :, 2:2 + W], in0=tb,
                                scalar1=-1.0, scalar2=None, op0=Alu.mult)

        # ---------------- mask pipeline stages ----------------
        def st_m2():
            nc.vector.tensor_tensor(
                out=m2[:, :, :, 0:PW - 1],
                in0=xb[:, :, :, 0:PW - 1], in1=xb[:, :, :, 1:PW], op=Alu.max)

        def st_m4():
            nc.vector.tensor_tensor(
                out=m4[:, :, :, 0:PW - 3],
                in0=m2[:, :, :, 0:PW - 3], in1=m2[:, :, :, 2:PW - 1],
                op=Alu.max)

        def st_hb():
            nc.vector.tensor_tensor(
                out=hb, in0=m4[:, :, :, 0:W], in1=xb[:, :, :, 4:4 + W],
                op=Alu.max)

        def st_shift():
            # border partitions keep the pre-set neutral values
            nc.gpsimd.memset(sh_dn[:, 0], -1.0)
            nc.gpsimd.memset(sh_dn[:, 1], -99.0)
            nc.gpsimd.memset(sh_up[:, 0], -1.0)
            nc.gpsimd.memset(sh_up[:, 1], -99.0)
            nc.gpsimd.dma_start(out=sh_dn[1:128], in_=hb[0:127, :, 2:4, :])
            nc.gpsimd.dma_start(out=sh_up[0:127], in_=hb[1:128, :, 0:2, :])

        def st_v1():
            nc.vector.tensor_tensor(out=va, in0=hb[:, :, 0, :],
                                    in1=hb[:, :, 1, :], op=Alu.max)
            nc.vector.tensor_tensor(out=vb, in0=hb[:, :, 2, :],
                                    in1=hb[:, :, 3, :], op=Alu.max)
            nc.vector.tensor_tensor(out=vm4, in0=va, in1=vb, op=Alu.max)

        def st_v2():
            nc.vector.tensor_tensor(out=vm012, in0=va, in1=hb[:, :, 2, :],
                                    op=Alu.max)
            nc.vector.tensor_tensor(out=vm123, in0=hb[:, :, 1, :], in1=vb,
                                    op=Alu.max)

        def st_v3():
            nc.vector.tensor_tensor(out=vx1, in0=sh_dn[:, :, 0, :],
                                    in1=sh_dn[:, :, 1, :], op=Alu.max)
            nc.vector.tensor_tensor(out=v[:, :, 0, :], in0=vx1, in1=vm012,
                                    op=Alu.max)
            nc.vector.tensor_tensor(out=v[:, :, 1, :], in0=sh_dn[:, :, 1, :],
                                    in1=vm4, op=Alu.max)

        def st_v4():
            nc.vector.tensor_tensor(out=v[:, :, 2, :], in0=vm4,
                                    in1=sh_up[:, :, 0, :], op=Alu.max)
            nc.vector.tensor_tensor(out=vx1, in0=sh_up[:, :, 0, :],
                                    in1=sh_up[:, :, 1, :], op=Alu.max)
            nc.vector.tensor_tensor(out=v[:, :, 3, :], in0=vx1, in1=vm123,
                                    op=Alu.max)

        def st_mask():
            # mask = (vmax != -vneg); accum_out collects msum per partition
            nc.vector.scalar_tensor_tensor(
                out=maskb, in0=v[:, 1, :, :], scalar=-1.0,
                in1=v[:, 0, :, :], op0=Alu.mult, op1=Alu.not_equal,
                accum_out=stc[:, 8:9])

        stages = [st_m2, st_m4, st_hb, st_shift, st_v1, st_v2, st_v3,
                  st_v4, st_mask]

        # ---------------- class loop, stages interleaved ----------------
        chunk_starts = list(range(0, C - 1, CHUNK)) + [C - 1]
        nchunks = len(chunk_starts)
        for k, c0 in enumerate(chunk_starts):
            nct = min(CHUNK, C - c0)
            p_t = ppool.tile([128, nct, G4, W], dt.float32, tag="p")
            nc.sync.dma_start(
                out=p_t,
                in_=pred.ap()[c0:c0 + nct].rearrange(
                    "c (p r) w -> p c r w", p=128))
            e_t = epool.tile([128, nct, G4, W], dt.float16, tag="e")
            nc.scalar.activation(out=e_t, in_=p_t, func=Act.Exp)
            # S path: pair-sum classes on DVE, then 4 PSUM-accum matmuls
            if nct == 2:
                e2 = e2pool.tile([128, G4, W], dt.float16, tag="e2")
                nc.vector.tensor_tensor(out=e2, in0=e_t[:, 0], in1=e_t[:, 1],
                                        op=Alu.add)
                src = e2
            else:
                src = e_t[:, 0]
            for j in range(G4):
                nc.tensor.matmul(s_ps[:, j, :], ident, src[:, j, :],
                                 start=(k == 0), stop=(k == nchunks - 1))
            # G path: one fused (t==c)*e_c per class
            for i in range(nct):
                c = c0 + i
                o_t = opool.tile([128, G4, W], dt.float16, tag="o")
                nc.vector.scalar_tensor_tensor(
                    out=o_t, in0=tb, scalar=float(c), in1=e_t[:, i],
                    op0=Alu.is_equal, op1=Alu.mult)
                for j in range(G4):
                    nc.tensor.matmul(g_ps[:, j, :], ident, o_t[:, j, :],
                                     start=(c == 0), stop=(c == C - 1))
            if 1 <= k <= len(stages):
                stages[k - 1]()

        # ---------------- finals (per row group) ----------------
        l1 = fin.tile([128, G4, W], dt.float16)
        l2 = fin.tile([128, G4, W], dt.float16)
        junk = fin.tile([128, W], dt.float16)
        for j in range(G4):
            nc.scalar.activation(out=l1[:, j, :], in_=s_ps[:, j, :],
                                 func=Act.Ln)
            nc.vector.scalar_tensor_tensor(
                out=junk, in0=l1[:, j, :], scalar=0.0, in1=maskb[:, j, :],
                op0=Alu.add, op1=Alu.mult, accum_out=stc[:, j:j + 1])
            nc.scalar.activation(out=l2[:, j, :], in_=g_ps[:, j, :],
                                 func=Act.Ln)
            nc.vector.scalar_tensor_tensor(
                out=junk, in0=l2[:, j, :], scalar=0.0, in1=maskb[:, j, :],
                op0=Alu.add, op1=Alu.mult, accum_out=stc[:, 4 + j:5 + j])

        # partition reduce via ones-matmul into the (consumed) S bank
        red = s_ps[0:1, 0, 0:12]
        nc.tensor.matmul(red, ones, stc[:, 0:12], start=True, stop=True)
        outsb = consts.tile([1, 16], dt.float32)
        nc.vector.memset(outsb, 0.0)
        nc.vector.tensor_copy(out=outsb[:, 0:12], in_=red)
        nc.sync.dma_start(out=out.ap(), in_=outsb)

    nc.compile()
    return nc


def get_nc():
    if "nc" not in _CACHE:
        _CACHE["nc"] = _build_nc()
    return _CACHE["nc"]


def _combine(outs):
    """outs: list of per-core [1,16] float32 -> scalar loss."""
    per_sample = []
    for o in outs:
        w1 = float(o[0, 0:4].sum())
        l2 = float(o[0, 4:8].sum())
        msum = float(o[0, 8])
        wsum = w1 - l2
        if msum > 0:
            per_sample.append(wsum / max(msum, 1.0))
        else:
            per_sample.append(wsum / float(H * W))
    return np.float32(np.mean(per_sample))


def kernel(pred, target):
    from concourse.bass_utils import run_bass_kernel_spmd

    pred = np.ascontiguousarray(pred, dtype=np.float32)
    target = np.ascontiguousarray(target, dtype=np.int32)
    assert pred.shape == (B, C, H, W) and target.shape == (B, H, W)

    nc = get_nc()
    in_maps = [{"pred": pred[b], "target": target[b]} for b in range(B)]
    res = run_bass_kernel_spmd(nc, in_maps, core_ids=list(range(N_CORES)))
    outs = [res.results[b]["out"] for b in range(B)]
    return np.asarray(_combine(outs), dtype=np.float32)
